# revision 32
# baseline (speedup 1.0000x reference)
"""Trainium2 Bass kernel for the masked depth-binned 3x3 conv (Conv2.5D).

Contract: kernel(**inputs) takes the FULL numpy inputs
  x     [8, 128, 64, 64] f32
  depth [8, 1, 64, 64]   f32
  fx    [8]              f32
  w0/w1/w2 [128, 128, 3, 3] f32
and returns the full output [8, 128, 64, 64] f32.

Strategy: data-parallel over N across the 8 NeuronCores (one sample per
core). Per core the op is decomposed as 25 shifted 1x1 matmuls
accumulated in PSUM, using a polynomial change of basis over the depth
bins instead of per-branch boolean masks:

The three depth bins are disjoint per (tap, pixel), with the neighbor
depth d falling in bin j = round((d-c)*fx/c) in {-1,0,+1} (or none).
Encoding the bin as a selector code sel = (2j+1)*inside in {0, 3, 1, -1},
every branch mask is a cubic polynomial in sel (all vanish at 0), so

  sum_b w_b @ (x . m_b)  ==  sum_{i=1..3} W'_i @ (x . sel^i)

with W'_i = sum_b c_{b,i} w_b folded into the weights on the host.
This removes all mask-bit computation (is_equal / ACT activations):
per tap the DVE only does three chained multiplies y_{i} = y_{i-1}*sel.
The center tap always has d == c (bin 1 exactly), so it is a single
unmasked matmul with w1 and needs no selector at all.
"""

import numpy as np

import concourse.bass as bass
import concourse.mybir as mybir
import concourse.bacc as bacc
import concourse.tile as tile
from concourse.bass_utils import run_bass_kernel_spmd

F32 = mybir.dt.float32
F16 = mybir.dt.float16
BF16 = mybir.dt.bfloat16
AF = mybir.ActivationFunctionType
OP = mybir.AluOpType

N, C, O, H, W = 8, 128, 128, 64, 64
L = H * W                    # 4096
PAD = 66                     # padded image row stride (66x66 image)
LP = PAD * PAD               # 4356
NT = 8                       # number of 512-wide output column tiles
NTW = L // NT                # 512
TAPS = (0, 1, 2, 3, 5, 6, 7, 8)   # non-center taps, matmul groups 1+3t+i
NG = 1 + 3 * len(TAPS)       # 25 matmul groups
MAGIC = 12582912.0           # 1.5 * 2^23: float32 round-to-nearest-int trick
# Lagrange coefficients of each branch mask on the basis {sel, sel^2, sel^3}
# for selector nodes (3, 1, -1) = branches (b0, b1, b2):
POLY = (
    (-1.0 / 24.0, 3.0 / 4.0, -3.0 / 8.0),   # sel^1 coeffs of (w0, w1, w2)
    (0.0, 1.0 / 2.0, 1.0 / 2.0),            # sel^2
    (1.0 / 24.0, -1.0 / 4.0, -1.0 / 8.0),   # sel^3
)
POOL_Y1 = ()                     # tap indices whose y1 multiply runs on GPSIMD
POOL_Y3 = ()                     # tap indices whose y3 also runs on GPSIMD


def _build_program(loop_n=None, ablate=()):
    """loop_n: if set, wrap the whole per-sample body in an on-device
    For_i loop (used only for timing measurements)."""
    nc = bacc.Bacc("TRN2", target_bir_lowering=False, debug=False)

    x_in = nc.dram_tensor("x_in", [C, L], F32, kind="ExternalInput")
    d_in = nc.dram_tensor("d_in", [H, W], F32, kind="ExternalInput")
    # receives fx (NOT 1/fx): selector ratio is (d-c) * fx * recip(c)
    fx_in = nc.dram_tensor("fx_in", [1, 1], F32, kind="ExternalInput")
    w_in = nc.dram_tensor("w_in", [NG, C, O], BF16, kind="ExternalInput")
    out_d = nc.dram_tensor("out", [O, L], F32, kind="ExternalOutput")

    with tile.TileContext(nc) as tc:
        with (
            tc.tile_pool(name="const", bufs=1) as cpool,
            tc.tile_pool(name="work", bufs=2) as wpool,
            tc.tile_pool(name="rowp", bufs=2, space="DRAM") as rowpool,
            tc.tile_pool(name="selp", bufs=5) as selpool,
            tc.tile_pool(name="ytil", bufs=3) as ypool,
            tc.tile_pool(name="psum", bufs=1, space="PSUM") as ppool,
        ):
          with (tc.For_i(0, loop_n, 1)
                if loop_n is not None
                else __import__("contextlib").nullcontext()):
              # ---- load & prep -------------------------------------------------
              # small control DMAs lead the SP queue so the selector chain
              # starts immediately; the big weight load rides the ACT queue
              # fx replicated across partitions by a broadcast DMA (keeps the
              # Pool engine free for the big x cast DMA)
              fx_col = cpool.tile([64, 1], F32, tag="fxcol")
              nc.sync.dma_start(
                  out=fx_col[:, :], in_=fx_in[0:1, :].partition_broadcast(64)
              )

              # the three row-shifted padded-depth views, loaded directly from
              # DRAM (drow[dy][p, c] = zero-padded d[p + dy - 1, c - 1]);
              # dy=1 (the center row, on the selector critical path) first
              drow = [None] * 3
              for dy in (1, 0, 2):
                  dr = cpool.tile([64, PAD], F32, tag=f"drow{dy}", name=f"drow{dy}")
                  nc.vector.memset(dr[:, :], 0.0)
                  r0, r1 = max(0, dy - 1), min(64, 63 + dy)
                  nc.sync.dma_start(
                      out=dr[r0 + 1 - dy : r1 + 1 - dy, 1:65], in_=d_in[r0:r1, :]
                  )
                  drow[dy] = dr

              # group-0 (center tap) weights early on the SP queue so the PE
              # can start as soon as the first half of x lands
              w_sb = cpool.tile([C, NG * O], BF16, tag="w")
              nc.sync.dma_start(out=w_sb[:, 0:O], in_=w_in[0, :, :])
              nc.scalar.dma_start(
                  out=w_sb[:, O:].rearrange("c (t o) -> c t o", t=NG - 1),
                  in_=w_in[1:, :, :].transpose([1, 0, 2]),
              )

              # padded fp16 activations; xb is xa shifted right by one element
              # so that odd-dx tap views stay 4-byte aligned (DVE 2x mode).
              xa = cpool.tile([C, LP], BF16, tag="xa")
              xb = cpool.tile([C, LP + 1], BF16, tag="xb")
              xa_r = xa[:, :].rearrange("c (r w) -> c r w", w=PAD)
              # zero only the padding border (interior is overwritten by the
              # casting DMA below)
              nc.vector.memset(xa[:, 0:PAD], 0.0)             # top row
              nc.vector.memset(xa[:, LP - PAD : LP], 0.0)     # bottom row
              nc.vector.memset(xa_r[:, 1:65, 0:1], 0.0)       # left col
              nc.vector.memset(xa_r[:, 1:65, 65:66], 0.0)     # right col
              # casting DMA (f32 dram -> fp16 sbuf); cast is SWDGE-only.
              # two halves so the first center-tap matmuls can start early
              x_r = x_in[:, :].rearrange("c (h w) -> c h w", w=W)
              nc.gpsimd.dma_start(out=xa_r[:, 1:33, 1:65], in_=x_r[:, 0:32, :])
              nc.gpsimd.dma_start(out=xa_r[:, 33:65, 1:65], in_=x_r[:, 32:64, :])
              nc.vector.memset(xb[:, 0:1], 0.0)
              # sbuf->sbuf copy on the ACT HWDGE queue to keep DVE free
              nc.scalar.dma_start(out=xb[:, 1 : LP + 1], in_=xa[:, :])
              xb_r = xb[:, 1 : LP + 1].rearrange("c (r w) -> c r w", w=PAD)

              # ---- selector: sel = (2*round(r)+1) * (r^2 <= 2.25) -------------
              # computed for all 9 taps at once in a compact [64, 576] layout
              cin = drow[1][:, 1:65]                        # center depth [64,64]
              rec = wpool.tile([64, 64], F32, tag="rec")
              nc.vector.reciprocal(rec[:, :], cin)
              ig = wpool.tile([64, 64], F32, tag="ig")      # fx / c
              nc.vector.tensor_scalar(
                  out=ig[:, :], in0=rec[:, :], scalar1=fx_col[:, :], scalar2=None,
                  op0=OP.mult,
              )
              dall = wpool.tile([64, 576], F32, tag="dall")
              for dy in range(3):
                  # the three dx-shifted [64,64] windows of this row as one
                  # overlapping-window AP read [[1,3],[1,64]]
                  nc.vector.tensor_copy(
                      out=dall[:, dy * 192 : (dy + 1) * 192].rearrange(
                          "p (t x) -> p t x", x=64
                      ),
                      in_=bass.AP(
                          drow[dy].tensor,
                          drow[dy][:, :].offset,
                          [[drow[dy][:, :].ap[0][0], 64], [1, 3], [1, 64]],
                      ),
                  )
              u = wpool.tile([64, 576], F32, tag="u")
              r = wpool.tile([64, 576], F32, tag="r")
              rr = wpool.tile([64, 576], F32, tag="rr")
              jj = wpool.tile([64, 576], F32, tag="jj")
              selk = wpool.tile([64, 576], BF16, tag="selk")

              def _rep(v, ntap):
                  # [64,64] AP broadcast along a stride-0 tap dim
                  return bass.AP(
                      v.tensor, v.offset, [[v.ap[0][0], 64], [0, ntap], [1, 64]]
                  )

              # two chunks so the first broadcasts can start halfway through
              rows9 = rowpool.tile([9, L], BF16, tag="rows9")
              rbase = rows9[:, :].offset
              selb = [None] * len(TAPS)

              def _sel_chunk(t0, t1):
                  c0, c1, nt = t0 * 64, t1 * 64, t1 - t0
                  sl = slice(c0, c1)
                  def r3(ap):
                      return ap.rearrange("p (t x) -> p t x", x=64)
                  nc.vector.tensor_tensor(
                      out=r3(u[:, sl]), in0=r3(dall[:, sl]),
                      in1=_rep(cin, nt), op=OP.subtract,
                  )
                  nc.vector.tensor_tensor(
                      out=r3(r[:, sl]), in0=r3(u[:, sl]),
                      in1=_rep(ig[:, :], nt), op=OP.mult,
                  )
                  # clamp to +-4 (guards inf from tiny center depths)
                  nc.vector.tensor_scalar(
                      out=r[:, sl], in0=r[:, sl], scalar1=-4.0, scalar2=4.0,
                      op0=OP.max, op1=OP.min,
                  )
                  nc.vector.tensor_tensor(
                      out=rr[:, sl], in0=r[:, sl], in1=r[:, sl], op=OP.mult
                  )
                  # j = round(r) via the magic-constant trick, then 2j+1
                  nc.vector.tensor_scalar(
                      out=jj[:, sl], in0=r[:, sl], scalar1=MAGIC, scalar2=MAGIC,
                      op0=OP.add, op1=OP.subtract,
                  )
                  nc.vector.tensor_scalar(
                      out=jj[:, sl], in0=jj[:, sl], scalar1=2.0, scalar2=1.0,
                      op0=OP.mult, op1=OP.add,
                  )
                  nc.vector.scalar_tensor_tensor(
                      out=selk[:, sl], in0=rr[:, sl], scalar=2.25, in1=jj[:, sl],
                      op0=OP.is_le, op1=OP.mult,
                  )
                  # flatten this chunk's taps: [64, nt*64] sbuf -> rows of
                  # [nt, L] dram (dram side iterated in (p, t, x) order)
                  nc.sync.dma_start(
                      out=bass.AP(
                          rows9.tensor, rbase + t0 * L, [[64, 64], [L, nt], [1, 64]]
                      ),
                      in_=selk[:, sl].rearrange("p (t x) -> p t x", x=64),
                  )
                  # per-tap broadcasts [1,L] -> [C,L] on the two HWDGE queues;
                  # tap 0 (the PE's first selector dependency) is split into
                  # two half-broadcasts running in parallel on both queues
                  for t, k in enumerate(TAPS):
                      if not (t0 <= k < t1):
                          continue
                      sb = selpool.tile([C, L], BF16, tag="sel", name=f"sel{t}")
                      if t == 0:
                          hl = L // 2
                          nc.sync.dma_start(
                              out=sb[:, 0:hl],
                              in_=rows9[k : k + 1, 0:hl].partition_broadcast(C),
                          )
                          nc.scalar.dma_start(
                              out=sb[:, hl:L],
                              in_=rows9[k : k + 1, hl:L].partition_broadcast(C),
                          )
                      else:
                          eng = (nc.sync, nc.scalar)[t % 2]
                          eng.dma_start(
                              out=sb[:, :],
                              in_=rows9[k : k + 1, :].partition_broadcast(C),
                          )
                      selb[t] = sb

              _sel_chunk(0, 1)
              _sel_chunk(1, 3)
              _sel_chunk(3, 6)
              _sel_chunk(6, 9)

              # ---- matmuls ----------------------------------------------------
              psums = [
                  ppool.tile([O, NTW], F32, tag=f"ps{t}", name=f"ps{t}")
                  for t in range(NT)
              ]

              nt_eff = 1 if "mm" in ablate else NT

              def mm_group(g, mv_ap):
                  # mv_ap: [C, L] flat tile or [C, 64, 64] row view
                  for t in range(nt_eff):
                      if len(mv_ap.ap) == 2:
                          sl = mv_ap[:, t * NTW : (t + 1) * NTW]
                      else:
                          sl = mv_ap[:, t * 8 : (t + 1) * 8, :]
                      nc.tensor.matmul(
                          psums[t][:, :],
                          w_sb[:, g * O : (g + 1) * O],
                          sl,
                          start=(g == 0),
                          stop=(g == NG - 1),
                      )

              # center tap first: plain w1 @ x, no selector dependency
              mm_group(0, xa_r[:, 1:65, 1:65])

              for t, k in enumerate(TAPS):
                  dy, dx = k // 3, k % 3
                  xsrc = xa_r if dx % 2 == 0 else xb_r
                  xview = xsrc[:, dy : dy + 64, dx : dx + 64]
                  sel_r = selb[t][:, :].rearrange("c (h w) -> c h w", w=W)
                  y1 = ypool.tile([C, L], BF16, tag="ya", name=f"y1_{t}")
                  y2 = ypool.tile([C, L], BF16, tag="yb", name=f"y2_{t}")
                  y3 = ypool.tile([C, L], BF16, tag="yc", name=f"y3_{t}")
                  # y1 has no DVE-chain dependency: offloaded taps compute it
                  # on GPSIMD well ahead of the DVE stream
                  if "mult" in ablate:
                      mm_group(1 + 3 * t, selb[t][:, :])
                      mm_group(2 + 3 * t, selb[t][:, :])
                      mm_group(3 + 3 * t, selb[t][:, :])
                      continue
                  y1eng = nc.gpsimd if t in POOL_Y1 else nc.vector
                  if t == 0:
                      # tap 0 in halves, each gated only on its half-broadcast,
                      # so the PE's first selector-dependent matmuls start early
                      for h2 in range(2):
                          rs = slice(32 * h2, 32 * (h2 + 1))
                          y1eng.tensor_tensor(
                              out=y1[:, :].rearrange("c (h w) -> c h w", w=W)[:, rs, :],
                              in0=xview[:, rs, :], in1=sel_r[:, rs, :], op=OP.mult,
                          )
                  else:
                      y1eng.tensor_tensor(
                          out=y1[:, :].rearrange("c (h w) -> c h w", w=W),
                          in0=xview, in1=sel_r, op=OP.mult,
                      )
                  mm_group(1 + 3 * t, y1[:, :])
                  nc.vector.tensor_tensor(
                      out=y2[:, :], in0=y1[:, :], in1=selb[t][:, :], op=OP.mult
                  )
                  mm_group(2 + 3 * t, y2[:, :])
                  y3eng = nc.gpsimd if t in POOL_Y3 else nc.vector
                  y3eng.tensor_tensor(
                      out=y3[:, :], in0=y2[:, :], in1=selb[t][:, :], op=OP.mult
                  )
                  mm_group(3 + 3 * t, y3[:, :])

              # ---- evict: ACT+DVE copy PSUM chunks in parallel, then chunked
              # stores on rotating queues overlap the remaining evictions
              osb = cpool.tile([O, L], F32, tag="osb")
              out_r = out_d[:, :].rearrange("o (t x) -> o t x", t=NT)
              for t in range(nt_eff):
                  ch = osb[:, t * NTW : (t + 1) * NTW]
                  # the tail-idle DVE and ACT evict alternating tiles
                  if t % 2 == 0:
                      nc.vector.tensor_copy(out=ch, in_=psums[t][:, :])
                  else:
                      nc.scalar.activation(out=ch, in_=psums[t][:, :], func=AF.Copy)
                  eng = (nc.sync, nc.scalar, nc.gpsimd)[t % 3]
                  eng.dma_start(out=out_r[:, t, :], in_=ch)

    nc.compile()
    return nc


_NC = None


def _get_program():
    global _NC
    if _NC is None:
        _NC = _build_program()
    return _NC


def _prep_weights(w0, w1, w2):
    """[NG, C(K), O(M)] bf16: group 0 = center-tap w1; groups 1+3t+i are
    the polynomial-basis recombinations W'_i for non-center tap t."""
    from ml_dtypes import bfloat16
    ws = [np.asarray(w, np.float64) for w in (w0, w1, w2)]
    wt = np.empty((NG, C, O), bfloat16)
    wt[0] = ws[1][:, :, 1, 1].T.astype(bfloat16)
    for t, k in enumerate(TAPS):
        dy, dx = k // 3, k % 3
        for i in range(3):
            c0, c1, c2 = POLY[i]
            Wi = c0 * ws[0][:, :, dy, dx] + c1 * ws[1][:, :, dy, dx] \
                + c2 * ws[2][:, :, dy, dx]
            wt[1 + 3 * t + i] = Wi.T.astype(bfloat16)
    return wt


def _make_in_maps(x, depth, fx, wt):
    return [
        {
            "x_in": np.ascontiguousarray(x[i].reshape(C, L)),
            "d_in": np.ascontiguousarray(depth[i, 0]),
            "fx_in": np.float32(fx[i]).reshape(1, 1),
            "w_in": wt,
        }
        for i in range(N)
    ]


def kernel(**inputs):
    x = np.ascontiguousarray(inputs["x"], np.float32)
    depth = np.ascontiguousarray(inputs["depth"], np.float32)
    fx = np.ascontiguousarray(inputs["fx"], np.float32)
    wt = _prep_weights(inputs["w0"], inputs["w1"], inputs["w2"])

    nc = _get_program()
    in_maps = _make_in_maps(x, depth, fx, wt)
    res = run_bass_kernel_spmd(nc, in_maps, core_ids=list(range(N)))
    out = np.stack([res.results[i]["out"] for i in range(N)])
    return out.reshape(N, O, H, W).astype(np.float32)


# revision 33
# speedup vs baseline: 1.0128x; 1.0128x over previous
"""Trainium2 Bass kernel for the masked depth-binned 3x3 conv (Conv2.5D).

Contract: kernel(**inputs) takes the FULL numpy inputs
  x     [8, 128, 64, 64] f32
  depth [8, 1, 64, 64]   f32
  fx    [8]              f32
  w0/w1/w2 [128, 128, 3, 3] f32
and returns the full output [8, 128, 64, 64] f32.

Strategy: data-parallel over N across the 8 NeuronCores (one sample per
core). Per core the op is decomposed as 25 shifted 1x1 matmuls
accumulated in PSUM, using a polynomial change of basis over the depth
bins instead of per-branch boolean masks:

The three depth bins are disjoint per (tap, pixel), with the neighbor
depth d falling in bin j = round((d-c)*fx/c) in {-1,0,+1} (or none).
Encoding the bin as a selector code sel = (2j+1)*inside in {0, 3, 1, -1},
every branch mask is a cubic polynomial in sel (all vanish at 0), so

  sum_b w_b @ (x . m_b)  ==  sum_{i=1..3} W'_i @ (x . sel^i)

with W'_i = sum_b c_{b,i} w_b folded into the weights on the host.
This removes all mask-bit computation (is_equal / ACT activations):
per tap the DVE only does three chained multiplies y_{i} = y_{i-1}*sel.
The center tap always has d == c (bin 1 exactly), so it is a single
unmasked matmul with w1 and needs no selector at all.
"""

import numpy as np

import concourse.bass as bass
import concourse.mybir as mybir
import concourse.bacc as bacc
import concourse.tile as tile
from concourse.bass_utils import run_bass_kernel_spmd

F32 = mybir.dt.float32
F16 = mybir.dt.float16
BF16 = mybir.dt.bfloat16
AF = mybir.ActivationFunctionType
OP = mybir.AluOpType

N, C, O, H, W = 8, 128, 128, 64, 64
L = H * W                    # 4096
PAD = 66                     # padded image row stride (66x66 image)
LP = PAD * PAD               # 4356
NT = 8                       # number of 512-wide output column tiles
NTW = L // NT                # 512
TAPS = (0, 1, 2, 3, 5, 6, 7, 8)   # non-center taps, matmul groups 1+3t+i
NG = 1 + 3 * len(TAPS)       # 25 matmul groups
MAGIC = 12582912.0           # 1.5 * 2^23: float32 round-to-nearest-int trick
# Lagrange coefficients of each branch mask on the basis {sel, sel^2, sel^3}
# for selector nodes (3, 1, -1) = branches (b0, b1, b2):
POLY = (
    (-1.0 / 24.0, 3.0 / 4.0, -3.0 / 8.0),   # sel^1 coeffs of (w0, w1, w2)
    (0.0, 1.0 / 2.0, 1.0 / 2.0),            # sel^2
    (1.0 / 24.0, -1.0 / 4.0, -1.0 / 8.0),   # sel^3
)
POOL_Y1 = ()                     # tap indices whose y1 multiply runs on GPSIMD
POOL_Y3 = ()                     # tap indices whose y3 also runs on GPSIMD


def _build_program(loop_n=None, ablate=()):
    """loop_n: if set, wrap the whole per-sample body in an on-device
    For_i loop (used only for timing measurements)."""
    nc = bacc.Bacc("TRN2", target_bir_lowering=False, debug=False)

    x_in = nc.dram_tensor("x_in", [C, L], F32, kind="ExternalInput")
    d_in = nc.dram_tensor("d_in", [H, W], F32, kind="ExternalInput")
    # receives fx (NOT 1/fx): selector ratio is (d-c) * fx * recip(c)
    fx_in = nc.dram_tensor("fx_in", [1, 1], F32, kind="ExternalInput")
    w_in = nc.dram_tensor("w_in", [NG, C, O], BF16, kind="ExternalInput")
    out_d = nc.dram_tensor("out", [O, L], F32, kind="ExternalOutput")

    with tile.TileContext(nc) as tc:
        with (
            tc.tile_pool(name="const", bufs=1) as cpool,
            tc.tile_pool(name="work", bufs=2) as wpool,
            tc.tile_pool(name="rowp", bufs=2, space="DRAM") as rowpool,
            tc.tile_pool(name="selp", bufs=4) as selpool,
            tc.tile_pool(name="ytil", bufs=2) as ypool,
            tc.tile_pool(name="psum", bufs=1, space="PSUM") as ppool,
        ):
          with (tc.For_i(0, loop_n, 1)
                if loop_n is not None
                else __import__("contextlib").nullcontext()):
              # ---- load & prep -------------------------------------------------
              # small control DMAs lead the SP queue so the selector chain
              # starts immediately; the big weight load rides the ACT queue
              # fx replicated across partitions by a broadcast DMA (keeps the
              # Pool engine free for the big x cast DMA)
              fx_col = cpool.tile([64, 1], F32, tag="fxcol")
              nc.sync.dma_start(
                  out=fx_col[:, :], in_=fx_in[0:1, :].partition_broadcast(64)
              )

              # the three row-shifted padded-depth views, loaded directly from
              # DRAM (drow[dy][p, c] = zero-padded d[p + dy - 1, c - 1]);
              # dy=1 (the center row, on the selector critical path) first
              drow = [None] * 3
              for dy in (1, 0, 2):
                  dr = cpool.tile([64, PAD], F32, tag=f"drow{dy}", name=f"drow{dy}")
                  nc.vector.memset(dr[:, :], 0.0)
                  r0, r1 = max(0, dy - 1), min(64, 63 + dy)
                  nc.sync.dma_start(
                      out=dr[r0 + 1 - dy : r1 + 1 - dy, 1:65], in_=d_in[r0:r1, :]
                  )
                  drow[dy] = dr

              # group-0 (center tap) weights early on the SP queue so the PE
              # can start as soon as the first half of x lands
              w_sb = cpool.tile([C, NG * O], BF16, tag="w")
              nc.sync.dma_start(out=w_sb[:, 0:O], in_=w_in[0, :, :])
              nc.scalar.dma_start(
                  out=w_sb[:, O:].rearrange("c (t o) -> c t o", t=NG - 1),
                  in_=w_in[1:, :, :].transpose([1, 0, 2]),
              )

              # padded fp16 activations; xb is xa shifted right by one element
              # so that odd-dx tap views stay 4-byte aligned (DVE 2x mode).
              xa = cpool.tile([C, LP], BF16, tag="xa")
              xb = cpool.tile([C, LP + 1], BF16, tag="xb")
              xa_r = xa[:, :].rearrange("c (r w) -> c r w", w=PAD)
              # zero only the padding border (interior is overwritten by the
              # casting DMA below)
              nc.vector.memset(xa[:, 0:PAD], 0.0)             # top row
              nc.vector.memset(xa[:, LP - PAD : LP], 0.0)     # bottom row
              nc.vector.memset(xa_r[:, 1:65, 0:1], 0.0)       # left col
              nc.vector.memset(xa_r[:, 1:65, 65:66], 0.0)     # right col
              # casting DMA (f32 dram -> fp16 sbuf); cast is SWDGE-only.
              # two halves so the first center-tap matmuls can start early
              x_r = x_in[:, :].rearrange("c (h w) -> c h w", w=W)
              nc.gpsimd.dma_start(out=xa_r[:, 1:33, 1:65], in_=x_r[:, 0:32, :])
              nc.gpsimd.dma_start(out=xa_r[:, 33:65, 1:65], in_=x_r[:, 32:64, :])
              nc.vector.memset(xb[:, 0:1], 0.0)
              # sbuf->sbuf copy on the ACT HWDGE queue to keep DVE free
              nc.scalar.dma_start(out=xb[:, 1 : LP + 1], in_=xa[:, :])
              xb_r = xb[:, 1 : LP + 1].rearrange("c (r w) -> c r w", w=PAD)

              # ---- selector: sel = (2*round(r)+1) * (r^2 <= 2.25) -------------
              # computed for all 9 taps at once in a compact [64, 576] layout
              cin = drow[1][:, 1:65]                        # center depth [64,64]
              rec = wpool.tile([64, 64], F32, tag="rec")
              nc.vector.reciprocal(rec[:, :], cin)
              ig = wpool.tile([64, 64], F32, tag="ig")      # fx / c
              nc.vector.tensor_scalar(
                  out=ig[:, :], in0=rec[:, :], scalar1=fx_col[:, :], scalar2=None,
                  op0=OP.mult,
              )
              dall = wpool.tile([64, 576], F32, tag="dall")
              for dy in range(3):
                  # the three dx-shifted [64,64] windows of this row as one
                  # overlapping-window AP read [[1,3],[1,64]]
                  nc.vector.tensor_copy(
                      out=dall[:, dy * 192 : (dy + 1) * 192].rearrange(
                          "p (t x) -> p t x", x=64
                      ),
                      in_=bass.AP(
                          drow[dy].tensor,
                          drow[dy][:, :].offset,
                          [[drow[dy][:, :].ap[0][0], 64], [1, 3], [1, 64]],
                      ),
                  )
              u = wpool.tile([64, 576], F32, tag="u")
              r = wpool.tile([64, 576], F32, tag="r")
              rr = wpool.tile([64, 576], F32, tag="rr")
              jj = wpool.tile([64, 576], F32, tag="jj")
              selk = wpool.tile([64, 576], BF16, tag="selk")

              def _rep(v, ntap):
                  # [64,64] AP broadcast along a stride-0 tap dim
                  return bass.AP(
                      v.tensor, v.offset, [[v.ap[0][0], 64], [0, ntap], [1, 64]]
                  )

              # two chunks so the first broadcasts can start halfway through
              rows9 = rowpool.tile([9, L], BF16, tag="rows9")
              rbase = rows9[:, :].offset
              selb = [None] * len(TAPS)

              def _sel_chunk(t0, t1):
                  c0, c1, nt = t0 * 64, t1 * 64, t1 - t0
                  sl = slice(c0, c1)
                  def r3(ap):
                      return ap.rearrange("p (t x) -> p t x", x=64)
                  nc.vector.tensor_tensor(
                      out=r3(u[:, sl]), in0=r3(dall[:, sl]),
                      in1=_rep(cin, nt), op=OP.subtract,
                  )
                  nc.vector.tensor_tensor(
                      out=r3(r[:, sl]), in0=r3(u[:, sl]),
                      in1=_rep(ig[:, :], nt), op=OP.mult,
                  )
                  # clamp to +-4 (guards inf from tiny center depths)
                  nc.vector.tensor_scalar(
                      out=r[:, sl], in0=r[:, sl], scalar1=-4.0, scalar2=4.0,
                      op0=OP.max, op1=OP.min,
                  )
                  nc.vector.tensor_tensor(
                      out=rr[:, sl], in0=r[:, sl], in1=r[:, sl], op=OP.mult
                  )
                  # j = round(r) via the magic-constant trick, then 2j+1
                  nc.vector.tensor_scalar(
                      out=jj[:, sl], in0=r[:, sl], scalar1=MAGIC, scalar2=MAGIC,
                      op0=OP.add, op1=OP.subtract,
                  )
                  nc.vector.tensor_scalar(
                      out=jj[:, sl], in0=jj[:, sl], scalar1=2.0, scalar2=1.0,
                      op0=OP.mult, op1=OP.add,
                  )
                  nc.vector.scalar_tensor_tensor(
                      out=selk[:, sl], in0=rr[:, sl], scalar=2.25, in1=jj[:, sl],
                      op0=OP.is_le, op1=OP.mult,
                  )
                  # flatten this chunk's taps: [64, nt*64] sbuf -> rows of
                  # [nt, L] dram (dram side iterated in (p, t, x) order)
                  nc.sync.dma_start(
                      out=bass.AP(
                          rows9.tensor, rbase + t0 * L, [[64, 64], [L, nt], [1, 64]]
                      ),
                      in_=selk[:, sl].rearrange("p (t x) -> p t x", x=64),
                  )
                  # per-tap broadcasts [1,L] -> [C,L] on the two HWDGE queues;
                  # tap 0 (the PE's first selector dependency) is split into
                  # two half-broadcasts running in parallel on both queues
                  for t, k in enumerate(TAPS):
                      if not (t0 <= k < t1):
                          continue
                      sb = selpool.tile([C, L], BF16, tag="sel", name=f"sel{t}")
                      if t == 0:
                          hl = L // 2
                          nc.sync.dma_start(
                              out=sb[:, 0:hl],
                              in_=rows9[k : k + 1, 0:hl].partition_broadcast(C),
                          )
                          nc.scalar.dma_start(
                              out=sb[:, hl:L],
                              in_=rows9[k : k + 1, hl:L].partition_broadcast(C),
                          )
                      else:
                          eng = (nc.sync, nc.scalar)[t % 2]
                          eng.dma_start(
                              out=sb[:, :],
                              in_=rows9[k : k + 1, :].partition_broadcast(C),
                          )
                      selb[t] = sb

              _sel_chunk(0, 1)
              _sel_chunk(1, 3)
              _sel_chunk(3, 6)
              _sel_chunk(6, 9)

              # ---- matmuls ----------------------------------------------------
              psums = [
                  ppool.tile([O, NTW], F32, tag=f"ps{t}", name=f"ps{t}")
                  for t in range(NT)
              ]

              nt_eff = 1 if "mm" in ablate else NT

              def mm_group(g, mv_ap):
                  # mv_ap: [C, L] flat tile or [C, 64, 64] row view
                  for t in range(nt_eff):
                      if len(mv_ap.ap) == 2:
                          sl = mv_ap[:, t * NTW : (t + 1) * NTW]
                      else:
                          sl = mv_ap[:, t * 8 : (t + 1) * 8, :]
                      nc.tensor.matmul(
                          psums[t][:, :],
                          w_sb[:, g * O : (g + 1) * O],
                          sl,
                          start=(g == 0),
                          stop=(g == NG - 1),
                      )

              # center tap first: plain w1 @ x, no selector dependency
              mm_group(0, xa_r[:, 1:65, 1:65])

              for t, k in enumerate(TAPS):
                  dy, dx = k // 3, k % 3
                  xsrc = xa_r if dx % 2 == 0 else xb_r
                  xview = xsrc[:, dy : dy + 64, dx : dx + 64]
                  sel_r = selb[t][:, :].rearrange("c (h w) -> c h w", w=W)
                  y1 = ypool.tile([C, L], BF16, tag="ya", name=f"y1_{t}")
                  y2 = ypool.tile([C, L], BF16, tag="yb", name=f"y2_{t}")
                  y3 = ypool.tile([C, L], BF16, tag="yc", name=f"y3_{t}")
                  # y1 has no DVE-chain dependency: offloaded taps compute it
                  # on GPSIMD well ahead of the DVE stream
                  if "mult" in ablate:
                      mm_group(1 + 3 * t, selb[t][:, :])
                      mm_group(2 + 3 * t, selb[t][:, :])
                      mm_group(3 + 3 * t, selb[t][:, :])
                      continue
                  y1eng = nc.gpsimd if t in POOL_Y1 else nc.vector
                  if t == 0:
                      # tap 0 in halves, each gated only on its half-broadcast,
                      # so the PE's first selector-dependent matmuls start early
                      for h2 in range(2):
                          rs = slice(32 * h2, 32 * (h2 + 1))
                          y1eng.tensor_tensor(
                              out=y1[:, :].rearrange("c (h w) -> c h w", w=W)[:, rs, :],
                              in0=xview[:, rs, :], in1=sel_r[:, rs, :], op=OP.mult,
                          )
                  else:
                      y1eng.tensor_tensor(
                          out=y1[:, :].rearrange("c (h w) -> c h w", w=W),
                          in0=xview, in1=sel_r, op=OP.mult,
                      )
                  mm_group(1 + 3 * t, y1[:, :])
                  nc.vector.tensor_tensor(
                      out=y2[:, :], in0=y1[:, :], in1=selb[t][:, :], op=OP.mult
                  )
                  mm_group(2 + 3 * t, y2[:, :])
                  y3eng = nc.gpsimd if t in POOL_Y3 else nc.vector
                  y3eng.tensor_tensor(
                      out=y3[:, :], in0=y2[:, :], in1=selb[t][:, :], op=OP.mult
                  )
                  mm_group(3 + 3 * t, y3[:, :])

              # ---- evict: ACT+DVE copy PSUM chunks in parallel, then chunked
              # stores on rotating queues overlap the remaining evictions
              osb = cpool.tile([O, L], F32, tag="osb")
              out_r = out_d[:, :].rearrange("o (t x) -> o t x", t=NT)
              for t in range(nt_eff):
                  ch = osb[:, t * NTW : (t + 1) * NTW]
                  # the tail-idle DVE and ACT evict alternating tiles
                  if t % 2 == 0:
                      nc.vector.tensor_copy(out=ch, in_=psums[t][:, :])
                  else:
                      nc.scalar.activation(out=ch, in_=psums[t][:, :], func=AF.Copy)
                  eng = (nc.sync, nc.scalar, nc.gpsimd)[t % 3]
                  eng.dma_start(out=out_r[:, t, :], in_=ch)

    nc.compile()
    return nc


_NC = None


def _get_program():
    global _NC
    if _NC is None:
        _NC = _build_program()
    return _NC


def _prep_weights(w0, w1, w2):
    """[NG, C(K), O(M)] bf16: group 0 = center-tap w1; groups 1+3t+i are
    the polynomial-basis recombinations W'_i for non-center tap t."""
    from ml_dtypes import bfloat16
    ws = [np.asarray(w, np.float64) for w in (w0, w1, w2)]
    wt = np.empty((NG, C, O), bfloat16)
    wt[0] = ws[1][:, :, 1, 1].T.astype(bfloat16)
    for t, k in enumerate(TAPS):
        dy, dx = k // 3, k % 3
        for i in range(3):
            c0, c1, c2 = POLY[i]
            Wi = c0 * ws[0][:, :, dy, dx] + c1 * ws[1][:, :, dy, dx] \
                + c2 * ws[2][:, :, dy, dx]
            wt[1 + 3 * t + i] = Wi.T.astype(bfloat16)
    return wt


def _make_in_maps(x, depth, fx, wt):
    return [
        {
            "x_in": np.ascontiguousarray(x[i].reshape(C, L)),
            "d_in": np.ascontiguousarray(depth[i, 0]),
            "fx_in": np.float32(fx[i]).reshape(1, 1),
            "w_in": wt,
        }
        for i in range(N)
    ]


def kernel(**inputs):
    x = np.ascontiguousarray(inputs["x"], np.float32)
    depth = np.ascontiguousarray(inputs["depth"], np.float32)
    fx = np.ascontiguousarray(inputs["fx"], np.float32)
    wt = _prep_weights(inputs["w0"], inputs["w1"], inputs["w2"])

    nc = _get_program()
    in_maps = _make_in_maps(x, depth, fx, wt)
    res = run_bass_kernel_spmd(nc, in_maps, core_ids=list(range(N)))
    out = np.stack([res.results[i]["out"] for i in range(N)])
    return out.reshape(N, O, H, W).astype(np.float32)
